# revision 1
# baseline (speedup 1.0000x reference)
"""Trainium2 Bass kernel for nn_Attention_81716047774180.

Dense transformer attention block:
  qkv = x @ qkv_w + qkv_b ; split into q,k,v heads [B,H,N,d]
  attn = softmax(q k^T * scale + rel_pos_bias) ; out = (attn @ v) @ proj_w + proj_b

Distribution: pure data-parallel over batch B=64 across 8 NeuronCores
(8 batches per core, no collectives).

Device algorithm (per core, bf16 compute, fp32 accumulation):
  - host folds: scale into q-weights, rel-index gather + exp + transpose into
    an expB table, x transposed to [DIM, tokens] so no on-device transposes.
  - optionally (USE_FP8) the q^T projection runs in fp8 e4m3 DoubleRow mode
    (K=256 per matmul, 2x PE throughput); host quantizes x and the scaled
    q-weights with power-of-two scales and the combined descale folds into
    the exp's scale argument for free.  k and v stay bf16 (error budget).
  - qk^T part computed transposed ([outdim, token]) so q^T,k^T land in [d, N]
    layout; v computed direct ([token, outdim]) with a ones column appended
    per head (gives softmax denominators for free from the P@V matmul).
  - S^T = k q^T per head ([nk, nq]) -> exp(scale*S) -> * expB -> P^T, so P@V
    needs no transposes: out^T[d, nq] = v^T P^T via lhsT=v.
  - denominators batched into a [12, N] reciprocal, broadcast over partitions
    with indicator-matrix matmuls, normalization as an in-place multiply on
    attn^T.
  - projection computed transposed: final^T[o, nq] += pw[:,o-tile]^T attn^T,
    staged per out-tile across batch pairs and written as out^T [DIM, toks]
    bf16 (host transposes back).  Pairing doubles the DMA descriptor size.
  - DMA pacing: only batch 0-2's x columns + weights + expB load up front;
    later token columns prefetch just-in-time inside the batch loop, keeping
    the HBM burst (and the package power controller) calm.
  - software pipeline over batches; the last batch gets an explicitly
    interleaved tail (group-wise denominator reciprocals + j-interleaved
    projection waves, with its elementwise work shifted to the otherwise
    idle GpSimd engine) to keep the PE fed during pipeline drain.
"""

import os
import sys

import numpy as np
import ml_dtypes

if "/opt/trn_rl_repo" not in sys.path:
    sys.path.insert(0, "/opt/trn_rl_repo")

B, N, DIM, H, d = 64, 320, 768, 12, 64
N_CORES = 8
B_LOC = B // N_CORES
KQ = DIM              # 768 q^T rows
NKC = [128, 128, 64]  # nk chunking of N=320
MT = [128, 128, 64]   # token chunking of N=320
VW = 65               # v columns per head incl. ones column

USE_FP8 = os.environ.get("USE_FP8", "0") == "1"
SX = 16.0             # fp8 scale on x
SWQ = 4096.0          # fp8 scale on (q-weights * d^-0.5)
# q_s = (x*SX) @ (wq*SWQ) is scaled by SX*SWQ; k is unscaled bf16, so
# S^T = k^T q_s needs exp(S_INV * S^T).
S_INV = 1.0 / (SX * SWQ) if USE_FP8 else 1.0

_BF16 = ml_dtypes.bfloat16
_E4M3 = ml_dtypes.float8_e4m3


def build_nc(n_batches=B_LOC, has_qkb=False, has_vb=False, has_pb=False):
    import concourse.bass as bass  # noqa: F401
    import concourse.tile as tile
    from concourse import bacc, mybir
    from contextlib import ExitStack

    bf16 = mybir.dt.bfloat16
    f8 = mybir.dt.float8e4
    f32 = mybir.dt.float32
    ACT = mybir.ActivationFunctionType
    DR = mybir.MatmulPerfMode.DoubleRow

    nc = bacc.Bacc("TRN2", target_bir_lowering=False, debug=False,
                   num_devices=N_CORES)

    toks = n_batches * N
    n_ktiles = 6 if USE_FP8 else 12
    if USE_FP8:
        x8_d = nc.dram_tensor("x8", [384, 2, toks], f8,
                              kind="ExternalInput").ap()
        wq8_d = nc.dram_tensor("wq8", [384, 2, KQ], f8,
                               kind="ExternalInput").ap()
    xT_d = nc.dram_tensor("xT", [DIM, toks], bf16, kind="ExternalInput").ap()
    wk_d = nc.dram_tensor("w_k", [DIM, 128 * n_ktiles], bf16,
                          kind="ExternalInput").ap()
    wv_d = nc.dram_tensor("w_v", [DIM, DIM], bf16, kind="ExternalInput").ap()
    pw_d = nc.dram_tensor("proj_w", [DIM, DIM], bf16, kind="ExternalInput").ap()
    eB_d = nc.dram_tensor("expB", [H, 128, 3, N], bf16,
                          kind="ExternalInput").ap()
    ind_d = nc.dram_tensor("ind", [H, DIM], bf16, kind="ExternalInput").ap()
    ind4_d = nc.dram_tensor("ind4", [4, 256], bf16, kind="ExternalInput").ap()
    if has_qkb:
        qkb_d = nc.dram_tensor("qkb", [12, 128], f32, kind="ExternalInput").ap()
    if has_vb:
        vb_d = nc.dram_tensor("vb", [1, DIM], bf16, kind="ExternalInput").ap()
    if has_pb:
        pb_d = nc.dram_tensor("pb", [1, DIM], bf16, kind="ExternalInput").ap()
    out_d = nc.dram_tensor("out", [DIM, toks], bf16, kind="ExternalOutput").ap()

    with tile.TileContext(nc) as tc, ExitStack() as ctx:
        sing = ctx.enter_context(tc.tile_pool(name="sing", bufs=1))
        qkT_p = ctx.enter_context(tc.tile_pool(name="qkT", bufs=2))
        v_p = ctx.enter_context(tc.tile_pool(name="v", bufs=2))
        pe_p = ctx.enter_context(tc.tile_pool(name="pe", bufs=4))
        pt_p = ctx.enter_context(tc.tile_pool(name="pt", bufs=76))
        rc_p = ctx.enter_context(tc.tile_pool(name="rc", bufs=2))
        aT_p = ctx.enter_context(tc.tile_pool(name="aT", bufs=2))
        o_p = ctx.enter_context(tc.tile_pool(name="o", bufs=1))
        # Separate PSUM pools so projection matmuls are not slot-blocked
        # behind attention tiles waiting on ACT exps (8 banks total).
        ps_s = ctx.enter_context(tc.tile_pool(name="ps_s", bufs=3, space="PSUM"))
        ps_o = ctx.enter_context(tc.tile_pool(name="ps_o", bufs=3, space="PSUM"))
        ps_g = ctx.enter_context(tc.tile_pool(name="ps_g", bufs=2, space="PSUM"))

        # ---- resident constants ----
        # DMA priority order: first q-units need only (wq8 + x8 | wk) and
        # batch-0 token columns; then expB; batches >= 3 token columns
        # prefetch just-in-time inside the batch loop to keep HBM power low.
        ind4_t = sing.tile([128, 256], bf16, tag="ind4")
        nc.sync.dma_start(ind4_t[:4, :], ind4_d[:, :])

        def filler(n=1):
            """Dependency-free bare weight loads that keep the PE pipeline
            ramped through DMA waits and pipeline drain (an idle PE drops
            to a half-speed p-state for ~3us after each gap).  Every real
            matmul emits its own LDWEIGHTS, so junk loads are overwritten
            before use and need no PSUM."""
            for _ in range(n):
                nc.tensor.ldweights(weights=ind4_t[0:4, 0:128])

        wq8_t = []
        x8_t = []
        wk_t = []
        wv_t = []
        pw_t = []
        xT_t = []
        t1 = min(3 * N, toks)
        if USE_FP8:
            for c in range(3):
                t = sing.tile([128, 2, KQ], f8, tag=f"wq8{c}", name=f"wq8{c}")
                nc.sync.dma_start(t[:], wq8_d[c * 128:(c + 1) * 128, :, :])
                wq8_t.append(t)
            for c in range(3):
                t = sing.tile([128, 2, toks], f8, tag=f"x8{c}", name=f"x8{c}")
                nc.sync.dma_start(t[:, :, 0:N],
                                  x8_d[c * 128:(c + 1) * 128, :, 0:N])
                x8_t.append(t)
        half = 64 * n_ktiles
        for kc in range(6):
            t = sing.tile([128, 128 * n_ktiles], bf16, tag=f"wk{kc}",
                          name=f"wk{kc}")
            nc.sync.dma_start(t[:, 0:half],
                              wk_d[kc * 128:(kc + 1) * 128, 0:half])
            wk_t.append(t)
        for kc in range(6):
            nc.sync.dma_start(wk_t[kc][:, half:],
                              wk_d[kc * 128:(kc + 1) * 128, half:])
        for kc in range(6):
            t = sing.tile([128, toks], bf16, tag=f"xT{kc}", name=f"xT{kc}")
            nc.sync.dma_start(t[:, 0:N], xT_d[kc * 128:(kc + 1) * 128, 0:N])
            xT_t.append(t)
        for kc in range(6):
            t = sing.tile([128, DIM], bf16, tag=f"wv{kc}", name=f"wv{kc}")
            nc.sync.dma_start(t[:], wv_d[kc * 128:(kc + 1) * 128, :])
            wv_t.append(t)
        if toks > N:
            if USE_FP8:
                for c in range(3):
                    nc.sync.dma_start(x8_t[c][:, :, N:t1],
                                      x8_d[c * 128:(c + 1) * 128, :, N:t1])
            for kc in range(6):
                nc.sync.dma_start(xT_t[kc][:, N:t1],
                                  xT_d[kc * 128:(kc + 1) * 128, N:t1])
        eB_t = []
        for h in range(H):
            t = sing.tile([128, 3, N], bf16, tag=f"eB{h}", name=f"eB{h}")
            nc.sync.dma_start(t[:], eB_d[h, :, :, :])
            eB_t.append(t)
        for kc in range(6):
            t = sing.tile([128, DIM], bf16, tag=f"pw{kc}", name=f"pw{kc}")
            nc.sync.dma_start(t[:], pw_d[kc * 128:(kc + 1) * 128, :])
            pw_t.append(t)
        ind_t = sing.tile([128, DIM], bf16, tag="ind")
        nc.sync.dma_start(ind_t[:H, :], ind_d[:, :])
        if has_qkb:
            qkb_t = sing.tile([128, 12], f32, tag="qkb")
            nc.sync.dma_start(qkb_t[:], qkb_d.rearrange("t p -> p t"))
        if has_vb or has_pb:
            ones_t = sing.tile([128, 128], bf16, tag="ones")
            nc.vector.memset(ones_t[:], 1.0)
        if has_vb:
            vb_t = sing.tile([1, DIM], bf16, tag="vb")
            nc.sync.dma_start(vb_t[:], vb_d[:, :])
        if has_pb:
            pb_t = sing.tile([1, DIM], bf16, tag="pb")
            nc.sync.dma_start(pb_t[:], pb_d[:, :])

        def prefetch(b):
            """Just-in-time DMA of batch b's token columns (issued ~3 batches
            ahead; queue FIFO order paces it behind earlier output DMAs)."""
            t0 = b * N
            if t0 < t1 or t0 >= toks:
                return
            if USE_FP8:
                for c in range(3):
                    nc.sync.dma_start(x8_t[c][:, :, t0:t0 + N],
                                      x8_d[c * 128:(c + 1) * 128, :, t0:t0 + N])
            for kc in range(6):
                nc.sync.dma_start(xT_t[kc][:, t0:t0 + N],
                                  xT_d[kc * 128:(kc + 1) * 128, t0:t0 + N])

        def qkv_units(b, boost=False):
            """q^T (fp8 DoubleRow or bf16) + k^T (bf16) + v projection units
            for batch b -> (units, state).  boost=True (prologue only):
            round-robin all three PSUM pools."""
            t0 = b * N
            qkT = [qkT_p.tile([128, N], bf16, tag=f"qkT{mt}", name=f"qkT{mt}")
                   for mt in range(12)]
            vt = [v_p.tile([128, H * VW], bf16, tag=f"v{mt}", name=f"v{mt}")
                  for mt in range(3)]
            pools = [ps_g, ps_s, ps_o] if boost else [ps_g]
            tags = ["psg", "pss", "pso"] if boost else ["psg"]
            pidx = [0]

            def pick():
                p, tg = pools[pidx[0] % len(pools)], tags[pidx[0] % len(tags)]
                pidx[0] += 1
                return p.tile([128, 384], f32, tag=tg, name=tg)

            def evict(mt, ps):
                if has_qkb:
                    nc.scalar.activation(qkT[mt][:], ps[:, :N], ACT.Copy,
                                         bias=qkb_t[:, mt:mt + 1])
                else:
                    nc.vector.tensor_copy(qkT[mt][:], ps[:, :N])

            def qT_unit(mt):
                # fp8 DoubleRow: contraction 256 per matmul, 3 chunks.
                ps = pick()
                for c in range(3):
                    nc.tensor.matmul(
                        ps[:, :N],
                        lhsT=wq8_t[c][:, :, mt * 128:(mt + 1) * 128],
                        rhs=x8_t[c][:, :, t0:t0 + N],
                        start=(c == 0), stop=(c == 2),
                        perf_mode=DR)
                evict(mt, ps)

            def kT_unit(mt):
                # covers q tiles too in the bf16 variant (mt 0..11)
                ps = pick()
                for kc in range(6):
                    nc.tensor.matmul(
                        ps[:, :N],
                        lhsT=wk_t[kc][:, mt * 128:(mt + 1) * 128],
                        rhs=xT_t[kc][:, t0:t0 + N],
                        start=(kc == 0), stop=(kc == 5))
                evict((6 if USE_FP8 else 0) + mt, ps)

            def v_unit(mt, nh):
                rows = MT[mt]
                t = vt[mt]
                ps = pick()
                for kc in range(6):
                    nc.tensor.matmul(
                        ps[:rows, :384],
                        lhsT=xT_t[kc][:, t0 + mt * 128:t0 + mt * 128 + rows],
                        rhs=wv_t[kc][:, nh * 384:(nh + 1) * 384],
                        start=(kc == 0), stop=(kc == 5 and not has_vb))
                if has_vb:
                    nc.tensor.matmul(
                        ps[:rows, :384],
                        lhsT=ones_t[0:1, 0:rows],
                        rhs=vb_t[0:1, nh * 384:(nh + 1) * 384],
                        start=False, stop=True)
                nc.scalar.activation(
                    t.rearrange("p (h c) -> p h c", c=VW)[:rows, nh * 6:(nh + 1) * 6, 0:64],
                    ps.rearrange("p (h c) -> p h c", c=64)[:rows, 0:6, :],
                    ACT.Copy)
                if nh == 1:
                    nc.vector.memset(
                        t.rearrange("p (h c) -> p h c", c=VW)[:rows, :, 64:65],
                        1.0)

            if USE_FP8:
                units = [lambda mt=mt: qT_unit(mt) for mt in range(6)]
                units += [lambda mt=mt: kT_unit(mt) for mt in range(6)]
            else:
                units = [lambda mt=mt: kT_unit(mt) for mt in range(12)]
            units += [lambda mt=mt, nh=nh: v_unit(mt, nh)
                      for mt in range(3) for nh in range(2)]
            return units, (qkT, vt)

        def score_units(b, state, pTs):
            """18 units, one per (pair, chunk): S^T of even+odd head + exp
            (fp8 descale folded into the scale argument) + bias-mul.
            The c<=1 bias-mul runs on the otherwise idle GpSimd engine."""
            qkT, vt = state

            def unit(j, c):
                ck = NKC[c]
                for r in range(2):
                    h = 2 * j + r
                    rb = r * 64
                    ps = ps_s.tile([128, N], f32, tag="pss", name="pss")
                    nc.tensor.matmul(
                        ps[:ck, :N],
                        lhsT=qkT[6 + j][rb:rb + 64, c * 128:c * 128 + ck],
                        rhs=qkT[j][rb:rb + 64, 0:N],
                        start=True, stop=True)
                    pexp = pe_p.tile([128, N], bf16, tag="pexp", name="pexp")
                    nc.scalar.activation(pexp[:ck, :], ps[:ck, :N], ACT.Exp,
                                         scale=S_INV)
                    pT = pt_p.tile([128, N], bf16, tag="pT", name="pT")
                    eng = nc.gpsimd if (c <= 1 or h < 6) else nc.vector
                    eng.tensor_mul(pT[:ck, :], pexp[:ck, :],
                                   eB_t[h][:ck, c, :])
                    pTs[h][c] = pT

            return [lambda j=j, c=c: unit(j, c)
                    for j in range(6) for c in range(3)]

        def pav(h, state, pTs, attnT, dstage, po, dens_dst, last=False):
            """P@V accumulation + denom extraction + unnorm evict for head h.
            dens_dst(f) -> (tile, base) where denominator group f lands
            (rows 0..3 at base 0 for the last batch; rows 4f..4f+3
            mid-stream).  The unnormalized attn^T eviction is spread
            ACT/DVE to keep both engines under the PE roofline."""
            qkT, vt = state
            j, r = divmod(h, 2)
            rb = r * 64
            for c in range(3):
                ck = NKC[c]
                nc.tensor.matmul(
                    po[0:VW, :N],
                    lhsT=vt[c][0:ck, h * VW:(h + 1) * VW],
                    rhs=pTs[h][c][0:ck, :],
                    start=(c == 0), stop=(c == 2))
            sr, sc_ = 32 * (h % 4), (h // 4) * N
            nc.vector.tensor_copy(dstage[sr:sr + 1, sc_:sc_ + N],
                                  po[64:65, :N])
            if h % 4 == 3:
                f = h // 4
                dt_, db = dens_dst(f)
                nc.sync.dma_start(
                    out=dt_[db:db + 4, :],
                    in_=dstage[0:128:32, f * N:(f + 1) * N])
            if last or h % 3 == 0:
                nc.scalar.activation(attnT[j][rb:rb + 64, :], po[0:64, :N],
                                     ACT.Copy)
            else:
                nc.vector.tensor_copy(attnT[j][rb:rb + 64, :], po[0:64, :N])

        def proj_mm(ps, j, attnT, ot, start, stop):
            nc.tensor.matmul(
                ps[:, :N],
                lhsT=pw_t[j][:, ot * 128:(ot + 1) * 128],
                rhs=attnT[j][:, :],
                start=start, stop=stop)

        def proj_fin(ps, ot, b, on_act=False):
            """Evict one out-tile into the batch-pair staging tile; DMA the
            pair's 640-wide slab once both halves are in (doubles the DMA
            descriptor size vs per-batch writes)."""
            if has_pb:
                nc.tensor.matmul(
                    ps[:, :N],
                    lhsT=pb_t[0:1, ot * 128:(ot + 1) * 128],
                    rhs=ones_t[0:1, 0:N],
                    start=False, stop=True)
            o_t = o_tiles[ot]
            half = b % 2
            if on_act:
                nc.scalar.activation(o_t[:, half * N:(half + 1) * N],
                                     ps[:, :N], ACT.Copy)
            else:
                nc.vector.tensor_copy(o_t[:, half * N:(half + 1) * N],
                                      ps[:, :N])
            if half == 1 or b == n_batches - 1:
                tp = (b // 2) * 2 * N
                w = (half + 1) * N
                nc.sync.dma_start(out_d[ot * 128:(ot + 1) * 128, tp:tp + w],
                                  o_t[:, 0:w])

        def av_units(b, state, pTs, attnT, dens, dstage):
            def unit(h):
                po = ps_o.tile([128, N], f32, tag="pso", name="pso")
                pav(h, state, pTs, attnT, dstage, po,
                    lambda f: (dens, 4 * f))

            return [lambda h=h: unit(h) for h in range(H)]

        def norm_proj_units(b, attnT, dens, denf, den_r):
            """Batched reciprocal, per-pair normalize, flipped projection."""

            def recip_unit():
                nc.vector.reciprocal_approx_fast(denf[:H, :], dens[:H, :])
                nc.scalar.activation(den_r[:H, :], denf[:H, :], ACT.Copy)

            def norm_unit(j):
                ps_b = ps_o.tile([128, N], f32, tag="pso", name="pso")
                nc.tensor.matmul(
                    ps_b[:, :N],
                    lhsT=ind_t[0:H, j * 128:(j + 1) * 128],
                    rhs=den_r[0:H, :],
                    start=True, stop=True)
                nc.vector.tensor_mul(attnT[j][:], attnT[j][:], ps_b[:, :N])

            def proj_unit(ot):
                ps = ps_g.tile([128, 384], f32, tag="psg", name="psg")
                for j in range(6):
                    proj_mm(ps, j, attnT, ot, j == 0, j == 5 and not has_pb)
                proj_fin(ps, ot, b)

            return ([recip_unit]
                    + [lambda j=j: norm_unit(j) for j in range(6)]
                    + [lambda ot=ot: proj_unit(ot) for ot in range(6)])

        def last_tail_units(b, state, pTs, attnT, dstage, p6):
            """Explicitly interleaved drain for the final batch: group-wise
            denominator reciprocals overlap later heads' P@V; projection
            accumulates per-j as soon as each attn^T pair is normalized;
            the previous batch's held-back projection units (p6) fill the
            dependency latencies.  Wave A = out-tiles {0,1} on ps_g, wave
            B = {2,3,4} on ps_s (idle after the last S^T), wave C = {5} on
            ps_g after A evicts; p6 slips into ps_g early / ps_s mid."""
            psA = [None, None]
            psB = [None, None, None]
            psC = [None]
            densL = [sing.tile([128, N], f32, tag=f"densL{f}",
                               name=f"densL{f}") for f in range(3)]
            denfL = [sing.tile([128, N], f32, tag=f"denfL{f}",
                               name=f"denfL{f}") for f in range(3)]
            denrL = [sing.tile([128, N], bf16, tag=f"denrL{f}",
                               name=f"denrL{f}") for f in range(3)]

            def av(h):
                pool = ps_s if h % 2 == 1 and h < 6 else ps_o
                po = pool.tile([128, N], f32,
                               tag="pss" if pool is ps_s else "pso",
                               name="pss" if pool is ps_s else "pso")
                pav(h, state, pTs, attnT, dstage, po,
                    lambda f: (densL[f], 0), last=True)

            def grec(f):
                nc.vector.reciprocal_approx_fast(denfL[f][0:4, :],
                                                 densL[f][0:4, :])
                nc.scalar.activation(denrL[f][0:4, :], denfL[f][0:4, :],
                                     ACT.Copy)

            def bm_nm(j):
                g, par = divmod(j, 2)
                ps_b = ps_o.tile([128, N], f32, tag="pso", name="pso")
                nc.tensor.matmul(
                    ps_b[:, :N],
                    lhsT=ind4_t[0:4, par * 128:(par + 1) * 128],
                    rhs=denrL[g][0:4, :],
                    start=True, stop=True)
                nc.vector.tensor_mul(attnT[j][:], attnT[j][:], ps_b[:, :N])

            def pjA(j):
                for i in range(2):
                    if j == 0:
                        psA[i] = ps_g.tile([128, 384], f32, tag="psg",
                                           name="psg")
                    proj_mm(psA[i], j, attnT, i, j == 0, j == 5 and not has_pb)
                    if j == 5:
                        proj_fin(psA[i], i, b)  # DVE: frees ps_g for wave C

            def pjB(j):
                for i in range(3):
                    if j == 0:
                        psB[i] = ps_s.tile([128, 384], f32, tag="pss",
                                           name="pss")
                    proj_mm(psB[i], j, attnT, 2 + i, j == 0,
                            j == 5 and not has_pb)
                    if j == 5:
                        proj_fin(psB[i], 2 + i, b, on_act=True)

            def pjC(j):
                if j == 0:
                    psC[0] = ps_g.tile([128, 384], f32, tag="psg", name="psg")
                proj_mm(psC[0], j, attnT, 5, j == 0, j == 5 and not has_pb)
                if j == 5:
                    proj_fin(psC[0], 5, b, on_act=True)

            A = lambda h: (lambda: av(h))          # noqa: E731
            G = lambda f: (lambda: grec(f))        # noqa: E731
            BN = lambda j: (lambda: bm_nm(j))      # noqa: E731
            PA = lambda j: (lambda: pjA(j))        # noqa: E731
            PB = lambda j: (lambda: pjB(j))        # noqa: E731
            PC = lambda j: (lambda: pjC(j))        # noqa: E731
            p6fn = p6 or (lambda ot: None)
            P6 = [(lambda ot=ot: p6fn(ot)) for ot in range(6)]
            return [
                A(0), P6[0], A(1), A(2), P6[1], A(3), G(0), A(4), P6[2],
                A(5), A(6), BN(0), PA(0), A(7), G(1), BN(1), PA(1),
                A(8), P6[3], A(9), BN(2), PA(2), A(10), P6[4], A(11), G(2),
                P6[5], BN(3), PA(3), PB(0), PB(1), PB(2), PB(3),
                BN(4), PA(4), PB(4), BN(5), PA(5), PB(5),
                PC(0), PC(1), PC(2), PC(3), PC(4), PC(5),
            ]

        o_tiles = [o_p.tile([128, 2 * N], bf16, tag=f"o{ot}", name=f"o{ot}")
                   for ot in range(6)]

        def _mk_held_factory(attnT, b):
            def run(ot):
                pool = ps_g if ot < 3 else ps_s
                ps = pool.tile([128, 384], f32,
                               tag="psg" if pool is ps_g else "pss",
                               name="psg" if pool is ps_g else "pss")
                for j in range(6):
                    proj_mm(ps, j, attnT, ot, j == 0, j == 5 and not has_pb)
                proj_fin(ps, ot, b)
            return run

        # Three-deep software pipeline over batches. Each step interleaves:
        #   - batch b's S^T/exp/bias-mul units   (PE + ACT/DVE wavefront)
        #   - batch b+1's qT/kT/v projection units (dense PE, independent)
        #   - batch b-1's P@V / normalize / proj (inputs all ready -> these
        #     fill every stall the exp wavefront would otherwise cause)
        # The projection of batch n-2 is held back as dependency-free PE
        # filler for the drain; ~60 junk weight-loads keep the PE clock
        # ramped through the initial DMA wait.
        qv_units, state = qkv_units(0, boost=True)
        for u in qv_units:
            u()
        prefetch(3)
        held_proj = None
        sc_last = None
        pTs_last = None
        tail = []          # av/norm/proj units of batch b-1
        for b in range(n_batches):
            attnT = [aT_p.tile([128, N], bf16, tag=f"aT{j}", name=f"aT{j}")
                     for j in range(6)]
            dens = rc_p.tile([128, N], f32, tag="dens", name="dens")
            dstage = rc_p.tile([128, 3 * N], f32, tag="dstage", name="dstage")
            denf = rc_p.tile([128, N], f32, tag="denf", name="denf")
            den_r = rc_p.tile([128, N], bf16, tag="den_r", name="den_r")
            pTs = [[None] * 3 for _ in range(H)]
            sc = score_units(b, state, pTs)
            if b + 1 < n_batches:
                qv, nstate = qkv_units(b + 1)
            else:
                qv, nstate = [], None
            ns, nq, nt = len(sc), len(qv), len(tail)
            for i in range(max(ns, nq, nt)):
                if i < nt:
                    tail[i]()
                if i < ns:
                    sc[i]()
                if i < nq:
                    qv[i]()
            if b == n_batches - 1 and n_batches > 1:
                tail = last_tail_units(b, state, pTs, attnT, dstage,
                                       held_proj)
            else:
                units = (av_units(b, state, pTs, attnT, dens, dstage)
                         + norm_proj_units(b, attnT, dens, denf, den_r))
                if b == n_batches - 2:
                    # hold this batch's projection back: it becomes the
                    # dependency-free PE filler for the drain
                    tail = units[:-6]
                    held_proj = _mk_held_factory(attnT, b)
                else:
                    tail = units
            state = nstate
            prefetch(b + 4)
        for u in tail:
            u()

    nc.compile()
    return nc


def prep_host(x, qkv_w, qkv_b, proj_w, proj_b, rpb_table, rel_index):
    """Host-side preprocessing: fold scale/gather/exp/transposes + fp8
    quantization of the q path."""
    scale = d ** -0.5
    qkv_w = np.asarray(qkv_w, np.float32)
    shared = {}
    if USE_FP8:
        wq = qkv_w[:, :DIM] * (scale * SWQ)
        np.clip(wq, -224.0, 224.0, out=wq)
        shared["wq8"] = np.ascontiguousarray(
            wq.astype(_E4M3).reshape(3, 2, 128, KQ).transpose(0, 2, 1, 3)
        ).reshape(384, 2, KQ)
        shared["w_k"] = np.ascontiguousarray(
            qkv_w[:, DIM:2 * DIM]).astype(_BF16)
    else:
        shared["w_k"] = np.ascontiguousarray(np.concatenate(
            [qkv_w[:, :DIM] * scale, qkv_w[:, DIM:2 * DIM]], axis=1)
        ).astype(_BF16)
    shared["w_v"] = np.ascontiguousarray(qkv_w[:, 2 * DIM:]).astype(_BF16)
    shared["proj_w"] = np.asarray(proj_w, np.float32).astype(_BF16)
    bias = np.asarray(rpb_table)[:, np.asarray(rel_index)]       # [H, nq, nk]
    expB = np.exp(bias.transpose(0, 2, 1)).astype(_BF16)          # [H, nk, nq]
    eBp = np.zeros((H, 384, N), dtype=_BF16)
    eBp[:, :N, :] = expB
    shared["expB"] = np.ascontiguousarray(
        eBp.reshape(H, 3, 128, N).transpose(0, 2, 1, 3))
    qkv_b = np.asarray(qkv_b, np.float32)
    qsc = scale * SX * SWQ if USE_FP8 else scale
    qkb = np.concatenate([qkv_b[:DIM] * qsc, qkv_b[DIM:2 * DIM]])
    vb = qkv_b[2 * DIM:]
    has_qkb = bool(np.any(qkb))
    has_vb = bool(np.any(vb))
    has_pb = bool(np.any(np.asarray(proj_b)))

    ind = np.zeros((H, DIM), dtype=_BF16)
    for h in range(H):
        ind[h, h * 64:(h + 1) * 64] = 1.0
    # ind4[r, par*128 + m] for the last batch's per-group broadcast:
    # pair j (parity par=j%2) takes den rows 2*par + (m>=64).
    ind4 = np.zeros((4, 256), dtype=_BF16)
    for par in range(2):
        ind4[2 * par, par * 128:par * 128 + 64] = 1.0
        ind4[2 * par + 1, par * 128 + 64:par * 128 + 128] = 1.0
    shared["ind"] = ind
    shared["ind4"] = ind4
    if has_qkb:
        shared["qkb"] = np.ascontiguousarray(qkb.reshape(12, 128)).astype(np.float32)
    if has_vb:
        shared["vb"] = vb.reshape(1, DIM).astype(_BF16)
    if has_pb:
        shared["pb"] = np.asarray(proj_b).reshape(1, DIM).astype(_BF16)

    in_maps = []
    for c in range(N_CORES):
        xs = np.asarray(x[c * B_LOC:(c + 1) * B_LOC], np.float32)
        xf = xs.reshape(B_LOC * N, DIM)
        xT = np.ascontiguousarray(xf.T).astype(_BF16)
        m = {"xT": xT}
        if USE_FP8:
            xq = xf.T * SX
            np.clip(xq, -224.0, 224.0, out=xq)
            m["x8"] = np.ascontiguousarray(
                xq.astype(_E4M3).reshape(3, 2, 128, B_LOC * N)
                .transpose(0, 2, 1, 3)).reshape(384, 2, B_LOC * N)
        m.update(shared)
        in_maps.append(m)
    return in_maps, has_qkb, has_vb, has_pb


_NC_CACHE = {}


def kernel(x, qkv_w, qkv_b, proj_w, proj_b, rpb_table, rel_index):
    from concourse.bass_utils import run_bass_kernel_spmd

    in_maps, has_qkb, has_vb, has_pb = prep_host(
        x, qkv_w, qkv_b, proj_w, proj_b, rpb_table, rel_index)
    key = (has_qkb, has_vb, has_pb, USE_FP8)
    if key not in _NC_CACHE:
        _NC_CACHE[key] = build_nc(B_LOC, has_qkb, has_vb, has_pb)
    nc = _NC_CACHE[key]
    res = run_bass_kernel_spmd(nc, in_maps, core_ids=list(range(N_CORES)))
    out = np.concatenate(
        [np.asarray(res.results[c]["out"], dtype=np.float32).T.reshape(
            B_LOC, N, DIM) for c in range(N_CORES)],
        axis=0)
    return out



# revision 5
# speedup vs baseline: 1.0188x; 1.0188x over previous
"""Trainium2 Bass kernel for nn_Attention_81716047774180.

Dense transformer attention block:
  qkv = x @ qkv_w + qkv_b ; split into q,k,v heads [B,H,N,d]
  attn = softmax(q k^T * scale + rel_pos_bias) ; out = (attn @ v) @ proj_w + proj_b

Distribution: pure data-parallel over batch B=64 across 8 NeuronCores
(8 batches per core, no collectives).

Device algorithm (per core, bf16 compute, fp32 accumulation):
  - host folds: scale into q-weights, rel-index gather + exp + transpose into
    an expB table, x transposed to [DIM, tokens] so no on-device transposes.
  - optionally (USE_FP8) the q^T projection runs in fp8 e4m3 DoubleRow mode
    (K=256 per matmul, 2x PE throughput); host quantizes x and the scaled
    q-weights with power-of-two scales and the combined descale folds into
    the exp's scale argument for free.  k and v stay bf16 (error budget).
  - qk^T part computed transposed ([outdim, token]) so q^T,k^T land in [d, N]
    layout; v computed direct ([token, outdim]) with a ones column appended
    per head (gives softmax denominators for free from the P@V matmul).
  - S^T = k q^T per head ([nk, nq]) -> exp(scale*S) -> * expB -> P^T, so P@V
    needs no transposes: out^T[d, nq] = v^T P^T via lhsT=v.
  - denominators batched into a [12, N] reciprocal, broadcast over partitions
    with indicator-matrix matmuls, normalization as an in-place multiply on
    attn^T.
  - projection computed transposed: final^T[o, nq] += pw[:,o-tile]^T attn^T,
    staged per out-tile across batch pairs and written as out^T [DIM, toks]
    bf16 (host transposes back).  Pairing doubles the DMA descriptor size.
  - DMA pacing: only batch 0-2's x columns + weights + expB load up front;
    later token columns prefetch just-in-time inside the batch loop, keeping
    the HBM burst (and the package power controller) calm.
  - software pipeline over batches; the last batch gets an explicitly
    interleaved tail (group-wise denominator reciprocals + j-interleaved
    projection waves, with its elementwise work shifted to the otherwise
    idle GpSimd engine) to keep the PE fed during pipeline drain.
"""

import os
import sys

import numpy as np
import ml_dtypes

if "/opt/trn_rl_repo" not in sys.path:
    sys.path.insert(0, "/opt/trn_rl_repo")

B, N, DIM, H, d = 64, 320, 768, 12, 64
N_CORES = 8
B_LOC = B // N_CORES
KQ = DIM              # 768 q^T rows
NKC = [128, 128, 64]  # nk chunking of N=320
MT = [128, 128, 64]   # token chunking of N=320
VW = 65               # v columns per head incl. ones column

USE_FP8 = os.environ.get("USE_FP8", "0") == "1"
SX = 16.0             # fp8 scale on x
SWQ = 4096.0          # fp8 scale on (q-weights * d^-0.5)
# q_s = (x*SX) @ (wq*SWQ) is scaled by SX*SWQ; k is unscaled bf16, so
# S^T = k^T q_s needs exp(S_INV * S^T).
S_INV = 1.0 / (SX * SWQ) if USE_FP8 else 1.0

_BF16 = ml_dtypes.bfloat16
_E4M3 = ml_dtypes.float8_e4m3


def build_nc(n_batches=B_LOC, has_qkb=False, has_vb=False, has_pb=False):
    import concourse.bass as bass  # noqa: F401
    import concourse.tile as tile
    from concourse import bacc, mybir
    from contextlib import ExitStack

    bf16 = mybir.dt.bfloat16
    f8 = mybir.dt.float8e4
    f32 = mybir.dt.float32
    ACT = mybir.ActivationFunctionType
    DR = mybir.MatmulPerfMode.DoubleRow

    nc = bacc.Bacc("TRN2", target_bir_lowering=False, debug=False,
                   num_devices=N_CORES)

    toks = n_batches * N
    n_ktiles = 6 if USE_FP8 else 12
    if USE_FP8:
        x8_d = nc.dram_tensor("x8", [384, 2, toks], f8,
                              kind="ExternalInput").ap()
        wq8_d = nc.dram_tensor("wq8", [384, 2, KQ], f8,
                               kind="ExternalInput").ap()
    xT_d = nc.dram_tensor("xT", [DIM, toks], bf16, kind="ExternalInput").ap()
    wk_d = nc.dram_tensor("w_k", [DIM, 128 * n_ktiles], bf16,
                          kind="ExternalInput").ap()
    wv_d = nc.dram_tensor("w_v", [DIM, DIM], bf16, kind="ExternalInput").ap()
    pw_d = nc.dram_tensor("proj_w", [DIM, DIM], bf16, kind="ExternalInput").ap()
    eB_d = nc.dram_tensor("expB", [H, 128, 3, N], bf16,
                          kind="ExternalInput").ap()
    ind_d = nc.dram_tensor("ind", [H, DIM], bf16, kind="ExternalInput").ap()
    ind4_d = nc.dram_tensor("ind4", [4, 256], bf16, kind="ExternalInput").ap()
    if has_qkb:
        qkb_d = nc.dram_tensor("qkb", [12, 128], f32, kind="ExternalInput").ap()
    if has_vb:
        vb_d = nc.dram_tensor("vb", [1, DIM], bf16, kind="ExternalInput").ap()
    if has_pb:
        pb_d = nc.dram_tensor("pb", [1, DIM], bf16, kind="ExternalInput").ap()
    out_d = nc.dram_tensor("out", [DIM, toks], bf16, kind="ExternalOutput").ap()

    with tile.TileContext(nc) as tc, ExitStack() as ctx:
        sing = ctx.enter_context(tc.tile_pool(name="sing", bufs=1))
        qkT_p = ctx.enter_context(tc.tile_pool(name="qkT", bufs=2))
        v_p = ctx.enter_context(tc.tile_pool(name="v", bufs=2))
        pe_p = ctx.enter_context(tc.tile_pool(name="pe", bufs=4))
        pt_p = ctx.enter_context(tc.tile_pool(name="pt", bufs=76))
        rc_p = ctx.enter_context(tc.tile_pool(name="rc", bufs=2))
        aT_p = ctx.enter_context(tc.tile_pool(name="aT", bufs=2))
        o_p = ctx.enter_context(tc.tile_pool(name="o", bufs=1))
        # Separate PSUM pools so projection matmuls are not slot-blocked
        # behind attention tiles waiting on ACT exps (8 banks total).
        ps_s = ctx.enter_context(tc.tile_pool(name="ps_s", bufs=3, space="PSUM"))
        ps_o = ctx.enter_context(tc.tile_pool(name="ps_o", bufs=3, space="PSUM"))
        ps_g = ctx.enter_context(tc.tile_pool(name="ps_g", bufs=2, space="PSUM"))

        # ---- resident constants ----
        # DMA priority order: first q-units need only (wq8 + x8 | wk) and
        # batch-0 token columns; then expB; batches >= 3 token columns
        # prefetch just-in-time inside the batch loop to keep HBM power low.
        ind4_t = sing.tile([128, 256], bf16, tag="ind4")
        nc.sync.dma_start(ind4_t[:4, :], ind4_d[:, :])

        def filler(n=1):
            """Dependency-free bare weight loads that keep the PE pipeline
            ramped through DMA waits and pipeline drain (an idle PE drops
            to a half-speed p-state for ~3us after each gap).  Every real
            matmul emits its own LDWEIGHTS, so junk loads are overwritten
            before use and need no PSUM."""
            for _ in range(n):
                nc.tensor.ldweights(weights=ind4_t[0:4, 0:128])

        wq8_t = []
        x8_t = []
        wk_t = []
        wv_t = []
        pw_t = []
        xT_t = []
        t1 = min(3 * N, toks)
        # x8 is a 4-batch ring buffer (slot = b % 4) so the fp8 copies fit
        # in SBUF alongside the bf16 xT tiles.
        x8_ring = min(4, n_batches)
        if USE_FP8:
            for c in range(3):
                t = sing.tile([128, 2, KQ], f8, tag=f"wq8{c}", name=f"wq8{c}")
                nc.sync.dma_start(t[:], wq8_d[c * 128:(c + 1) * 128, :, :])
                wq8_t.append(t)
            for c in range(3):
                t = sing.tile([128, 2, x8_ring * N], f8, tag=f"x8{c}",
                              name=f"x8{c}")
                nc.sync.dma_start(t[:, :, 0:N],
                                  x8_d[c * 128:(c + 1) * 128, :, 0:N])
                x8_t.append(t)
        half = 64 * n_ktiles
        for kc in range(6):
            t = sing.tile([128, 128 * n_ktiles], bf16, tag=f"wk{kc}",
                          name=f"wk{kc}")
            nc.sync.dma_start(t[:, 0:half],
                              wk_d[kc * 128:(kc + 1) * 128, 0:half])
            wk_t.append(t)
        for kc in range(6):
            nc.sync.dma_start(wk_t[kc][:, half:],
                              wk_d[kc * 128:(kc + 1) * 128, half:])
        for kc in range(6):
            t = sing.tile([128, toks], bf16, tag=f"xT{kc}", name=f"xT{kc}")
            nc.sync.dma_start(t[:, 0:N], xT_d[kc * 128:(kc + 1) * 128, 0:N])
            xT_t.append(t)
        for kc in range(6):
            t = sing.tile([128, DIM], bf16, tag=f"wv{kc}", name=f"wv{kc}")
            nc.sync.dma_start(t[:], wv_d[kc * 128:(kc + 1) * 128, :])
            wv_t.append(t)
        if toks > N:
            if USE_FP8:
                for c in range(3):
                    nc.sync.dma_start(x8_t[c][:, :, N:t1],
                                      x8_d[c * 128:(c + 1) * 128, :, N:t1])
                assert t1 <= x8_ring * N
            for kc in range(6):
                nc.sync.dma_start(xT_t[kc][:, N:t1],
                                  xT_d[kc * 128:(kc + 1) * 128, N:t1])
        eB_t = []
        for h in range(H):
            t = sing.tile([128, 3, N], bf16, tag=f"eB{h}", name=f"eB{h}")
            nc.sync.dma_start(t[:], eB_d[h, :, :, :])
            eB_t.append(t)
        for kc in range(6):
            t = sing.tile([128, DIM], bf16, tag=f"pw{kc}", name=f"pw{kc}")
            nc.sync.dma_start(t[:], pw_d[kc * 128:(kc + 1) * 128, :])
            pw_t.append(t)
        ind_t = sing.tile([128, DIM], bf16, tag="ind")
        nc.sync.dma_start(ind_t[:H, :], ind_d[:, :])
        if has_qkb:
            qkb_t = sing.tile([128, 12], f32, tag="qkb")
            nc.sync.dma_start(qkb_t[:], qkb_d.rearrange("t p -> p t"))
        if has_vb or has_pb:
            ones_t = sing.tile([128, 128], bf16, tag="ones")
            nc.vector.memset(ones_t[:], 1.0)
        if has_vb:
            vb_t = sing.tile([1, DIM], bf16, tag="vb")
            nc.sync.dma_start(vb_t[:], vb_d[:, :])
        if has_pb:
            pb_t = sing.tile([1, DIM], bf16, tag="pb")
            nc.sync.dma_start(pb_t[:], pb_d[:, :])

        def prefetch(b):
            """Just-in-time DMA of batch b's token columns (issued ~3 batches
            ahead; queue FIFO order paces it behind earlier output DMAs)."""
            t0 = b * N
            if t0 < t1 or t0 >= toks:
                return
            if USE_FP8:
                s0 = (t0 // N % x8_ring) * N
                for c in range(3):
                    nc.sync.dma_start(x8_t[c][:, :, s0:s0 + N],
                                      x8_d[c * 128:(c + 1) * 128, :, t0:t0 + N])
            for kc in range(6):
                nc.sync.dma_start(xT_t[kc][:, t0:t0 + N],
                                  xT_d[kc * 128:(kc + 1) * 128, t0:t0 + N])

        def qkv_units(b, boost=False):
            """q^T (fp8 DoubleRow or bf16) + k^T (bf16) + v projection units
            for batch b -> (units, state).  boost=True (prologue only):
            round-robin all three PSUM pools."""
            t0 = b * N
            qkT = [qkT_p.tile([128, N], bf16, tag=f"qkT{mt}", name=f"qkT{mt}")
                   for mt in range(12)]
            vt = [v_p.tile([128, H * VW], bf16, tag=f"v{mt}", name=f"v{mt}")
                  for mt in range(3)]
            pools = [ps_g, ps_s, ps_o] if boost else [ps_g]
            tags = ["psg", "pss", "pso"] if boost else ["psg"]
            pidx = [0]

            def pick():
                p, tg = pools[pidx[0] % len(pools)], tags[pidx[0] % len(tags)]
                pidx[0] += 1
                return p.tile([128, 384], f32, tag=tg, name=tg)

            def evict(mt, ps):
                if has_qkb:
                    nc.scalar.activation(qkT[mt][:], ps[:, :N], ACT.Copy,
                                         bias=qkb_t[:, mt:mt + 1])
                else:
                    nc.vector.tensor_copy(qkT[mt][:], ps[:, :N])

            def qT_unit(mt):
                # fp8 DoubleRow: contraction 256 per matmul, 3 chunks.
                s0 = (b % x8_ring) * N
                ps = pick()
                for c in range(3):
                    nc.tensor.matmul(
                        ps[:, :N],
                        lhsT=wq8_t[c][:, :, mt * 128:(mt + 1) * 128],
                        rhs=x8_t[c][:, :, s0:s0 + N],
                        start=(c == 0), stop=(c == 2),
                        perf_mode=DR)
                evict(mt, ps)

            def kT_unit(mt):
                # covers q tiles too in the bf16 variant (mt 0..11)
                ps = pick()
                for kc in range(6):
                    nc.tensor.matmul(
                        ps[:, :N],
                        lhsT=wk_t[kc][:, mt * 128:(mt + 1) * 128],
                        rhs=xT_t[kc][:, t0:t0 + N],
                        start=(kc == 0), stop=(kc == 5))
                evict((6 if USE_FP8 else 0) + mt, ps)

            def v_unit(mt, nh):
                rows = MT[mt]
                t = vt[mt]
                ps = pick()
                for kc in range(6):
                    nc.tensor.matmul(
                        ps[:rows, :384],
                        lhsT=xT_t[kc][:, t0 + mt * 128:t0 + mt * 128 + rows],
                        rhs=wv_t[kc][:, nh * 384:(nh + 1) * 384],
                        start=(kc == 0), stop=(kc == 5 and not has_vb))
                if has_vb:
                    nc.tensor.matmul(
                        ps[:rows, :384],
                        lhsT=ones_t[0:1, 0:rows],
                        rhs=vb_t[0:1, nh * 384:(nh + 1) * 384],
                        start=False, stop=True)
                nc.scalar.activation(
                    t.rearrange("p (h c) -> p h c", c=VW)[:rows, nh * 6:(nh + 1) * 6, 0:64],
                    ps.rearrange("p (h c) -> p h c", c=64)[:rows, 0:6, :],
                    ACT.Copy)
                if nh == 1:
                    nc.vector.memset(
                        t.rearrange("p (h c) -> p h c", c=VW)[:rows, :, 64:65],
                        1.0)

            if USE_FP8:
                units = [lambda mt=mt: qT_unit(mt) for mt in range(6)]
                units += [lambda mt=mt: kT_unit(mt) for mt in range(6)]
            else:
                units = [lambda mt=mt: kT_unit(mt) for mt in range(12)]
            units += [lambda mt=mt, nh=nh: v_unit(mt, nh)
                      for mt in range(3) for nh in range(2)]
            return units, (qkT, vt)

        def score_units(b, state, pTs):
            """18 units, one per (pair, chunk): S^T of even+odd head + exp
            (fp8 descale folded into the scale argument) + bias-mul.
            The c<=1 bias-mul runs on the otherwise idle GpSimd engine."""
            qkT, vt = state

            def unit(j, c):
                ck = NKC[c]
                for r in range(2):
                    h = 2 * j + r
                    rb = r * 64
                    ps = ps_s.tile([128, N], f32, tag="pss", name="pss")
                    nc.tensor.matmul(
                        ps[:ck, :N],
                        lhsT=qkT[6 + j][rb:rb + 64, c * 128:c * 128 + ck],
                        rhs=qkT[j][rb:rb + 64, 0:N],
                        start=True, stop=True)
                    pexp = pe_p.tile([128, N], bf16, tag="pexp", name="pexp")
                    nc.scalar.activation(pexp[:ck, :], ps[:ck, :N], ACT.Exp,
                                         scale=S_INV)
                    pT = pt_p.tile([128, N], bf16, tag="pT", name="pT")
                    eng = nc.gpsimd if (c <= 1 or h < 6) else nc.vector
                    eng.tensor_mul(pT[:ck, :], pexp[:ck, :],
                                   eB_t[h][:ck, c, :])
                    pTs[h][c] = pT

            return [lambda j=j, c=c: unit(j, c)
                    for j in range(6) for c in range(3)]

        def pav(h, state, pTs, attnT, dstage, po, dens_dst, last=False):
            """P@V accumulation + denom extraction + unnorm evict for head h.
            dens_dst(f) -> (tile, base) where denominator group f lands
            (rows 0..3 at base 0 for the last batch; rows 4f..4f+3
            mid-stream).  The unnormalized attn^T eviction is spread
            ACT/DVE to keep both engines under the PE roofline."""
            qkT, vt = state
            j, r = divmod(h, 2)
            rb = r * 64
            for c in range(3):
                ck = NKC[c]
                nc.tensor.matmul(
                    po[0:VW, :N],
                    lhsT=vt[c][0:ck, h * VW:(h + 1) * VW],
                    rhs=pTs[h][c][0:ck, :],
                    start=(c == 0), stop=(c == 2))
            sr, sc_ = 32 * (h % 4), (h // 4) * N
            nc.vector.tensor_copy(dstage[sr:sr + 1, sc_:sc_ + N],
                                  po[64:65, :N])
            if h % 4 == 3:
                f = h // 4
                dt_, db = dens_dst(f)
                nc.sync.dma_start(
                    out=dt_[db:db + 4, :],
                    in_=dstage[0:128:32, f * N:(f + 1) * N])
            if last or h % 3 == 0:
                nc.scalar.activation(attnT[j][rb:rb + 64, :], po[0:64, :N],
                                     ACT.Copy)
            else:
                nc.vector.tensor_copy(attnT[j][rb:rb + 64, :], po[0:64, :N])

        def proj_mm(ps, j, attnT, ot, start, stop):
            nc.tensor.matmul(
                ps[:, :N],
                lhsT=pw_t[j][:, ot * 128:(ot + 1) * 128],
                rhs=attnT[j][:, :],
                start=start, stop=stop)

        def proj_fin(ps, ot, b, on_act=False):
            """Evict one out-tile into the batch-pair staging tile; DMA the
            pair's 640-wide slab once both halves are in (doubles the DMA
            descriptor size vs per-batch writes)."""
            if has_pb:
                nc.tensor.matmul(
                    ps[:, :N],
                    lhsT=pb_t[0:1, ot * 128:(ot + 1) * 128],
                    rhs=ones_t[0:1, 0:N],
                    start=False, stop=True)
            o_t = o_tiles[ot]
            half = b % 2
            if on_act:
                nc.scalar.activation(o_t[:, half * N:(half + 1) * N],
                                     ps[:, :N], ACT.Copy)
            else:
                nc.vector.tensor_copy(o_t[:, half * N:(half + 1) * N],
                                      ps[:, :N])
            if half == 1 or b == n_batches - 1:
                tp = (b // 2) * 2 * N
                w = (half + 1) * N
                nc.sync.dma_start(out_d[ot * 128:(ot + 1) * 128, tp:tp + w],
                                  o_t[:, 0:w])

        def av_units(b, state, pTs, attnT, dens, dstage):
            def unit(h):
                po = ps_o.tile([128, N], f32, tag="pso", name="pso")
                pav(h, state, pTs, attnT, dstage, po,
                    lambda f: (dens, 4 * f))

            return [lambda h=h: unit(h) for h in range(H)]

        def norm_proj_units(b, attnT, dens, denf, den_r):
            """Batched reciprocal, per-pair normalize, flipped projection."""

            def recip_unit():
                nc.vector.reciprocal_approx_fast(denf[:H, :], dens[:H, :])
                nc.scalar.activation(den_r[:H, :], denf[:H, :], ACT.Copy)

            def norm_unit(j):
                ps_b = ps_o.tile([128, N], f32, tag="pso", name="pso")
                nc.tensor.matmul(
                    ps_b[:, :N],
                    lhsT=ind_t[0:H, j * 128:(j + 1) * 128],
                    rhs=den_r[0:H, :],
                    start=True, stop=True)
                nc.vector.tensor_mul(attnT[j][:], attnT[j][:], ps_b[:, :N])

            def proj_unit(ot):
                ps = ps_g.tile([128, 384], f32, tag="psg", name="psg")
                for j in range(6):
                    proj_mm(ps, j, attnT, ot, j == 0, j == 5 and not has_pb)
                proj_fin(ps, ot, b)

            return ([recip_unit]
                    + [lambda j=j: norm_unit(j) for j in range(6)]
                    + [lambda ot=ot: proj_unit(ot) for ot in range(6)])

        def last_tail_units(b, state, pTs, attnT, dstage, p6):
            """Explicitly interleaved drain for the final batch: group-wise
            denominator reciprocals overlap later heads' P@V; projection
            accumulates per-j as soon as each attn^T pair is normalized;
            the previous batch's held-back projection units (p6) fill the
            dependency latencies.  Wave A = out-tiles {0,1} on ps_g, wave
            B = {2,3,4} on ps_s (idle after the last S^T), wave C = {5} on
            ps_g after A evicts; p6 slips into ps_g early / ps_s mid."""
            psA = [None, None]
            psB = [None, None, None]
            psC = [None]
            densL = [sing.tile([128, N], f32, tag=f"densL{f}",
                               name=f"densL{f}") for f in range(3)]
            denfL = [sing.tile([128, N], f32, tag=f"denfL{f}",
                               name=f"denfL{f}") for f in range(3)]
            denrL = [sing.tile([128, N], bf16, tag=f"denrL{f}",
                               name=f"denrL{f}") for f in range(3)]

            def av(h):
                pool = ps_s if h % 2 == 1 and h < 6 else ps_o
                po = pool.tile([128, N], f32,
                               tag="pss" if pool is ps_s else "pso",
                               name="pss" if pool is ps_s else "pso")
                pav(h, state, pTs, attnT, dstage, po,
                    lambda f: (densL[f], 0), last=True)

            def grec(f):
                nc.vector.reciprocal_approx_fast(denfL[f][0:4, :],
                                                 densL[f][0:4, :])
                nc.scalar.activation(denrL[f][0:4, :], denfL[f][0:4, :],
                                     ACT.Copy)

            def bm_nm(j):
                g, par = divmod(j, 2)
                ps_b = ps_o.tile([128, N], f32, tag="pso", name="pso")
                nc.tensor.matmul(
                    ps_b[:, :N],
                    lhsT=ind4_t[0:4, par * 128:(par + 1) * 128],
                    rhs=denrL[g][0:4, :],
                    start=True, stop=True)
                nc.vector.tensor_mul(attnT[j][:], attnT[j][:], ps_b[:, :N])

            def pjA(j):
                for i in range(2):
                    if j == 0:
                        psA[i] = ps_g.tile([128, 384], f32, tag="psg",
                                           name="psg")
                    proj_mm(psA[i], j, attnT, i, j == 0, j == 5 and not has_pb)
                    if j == 5:
                        proj_fin(psA[i], i, b)  # DVE: frees ps_g for wave C

            def pjB(j):
                for i in range(3):
                    if j == 0:
                        psB[i] = ps_s.tile([128, 384], f32, tag="pss",
                                           name="pss")
                    proj_mm(psB[i], j, attnT, 2 + i, j == 0,
                            j == 5 and not has_pb)
                    if j == 5:
                        proj_fin(psB[i], 2 + i, b, on_act=True)

            def pjC(j):
                if j == 0:
                    psC[0] = ps_g.tile([128, 384], f32, tag="psg", name="psg")
                proj_mm(psC[0], j, attnT, 5, j == 0, j == 5 and not has_pb)
                if j == 5:
                    proj_fin(psC[0], 5, b, on_act=True)

            A = lambda h: (lambda: av(h))          # noqa: E731
            G = lambda f: (lambda: grec(f))        # noqa: E731
            BN = lambda j: (lambda: bm_nm(j))      # noqa: E731
            PA = lambda j: (lambda: pjA(j))        # noqa: E731
            PB = lambda j: (lambda: pjB(j))        # noqa: E731
            PC = lambda j: (lambda: pjC(j))        # noqa: E731
            p6fn = p6 or (lambda ot: None)
            P6 = [(lambda ot=ot: p6fn(ot)) for ot in range(6)]
            return [
                A(0), P6[0], A(1), A(2), P6[1], A(3), G(0), A(4), P6[2],
                A(5), A(6), BN(0), PA(0), A(7), G(1), BN(1), PA(1),
                A(8), P6[3], A(9), BN(2), PA(2), A(10), P6[4], A(11), G(2),
                P6[5], BN(3), PA(3), PB(0), PB(1), PB(2), PB(3),
                BN(4), PA(4), PB(4), BN(5), PA(5), PB(5),
                PC(0), PC(1), PC(2), PC(3), PC(4), PC(5),
            ]

        o_tiles = [o_p.tile([128, 2 * N], bf16, tag=f"o{ot}", name=f"o{ot}")
                   for ot in range(6)]

        def _mk_held_factory(attnT, b):
            def run(ot):
                pool = ps_g if ot < 3 else ps_s
                ps = pool.tile([128, 384], f32,
                               tag="psg" if pool is ps_g else "pss",
                               name="psg" if pool is ps_g else "pss")
                for j in range(6):
                    proj_mm(ps, j, attnT, ot, j == 0, j == 5 and not has_pb)
                proj_fin(ps, ot, b)
            return run

        # Three-deep software pipeline over batches. Each step interleaves:
        #   - batch b's S^T/exp/bias-mul units   (PE + ACT/DVE wavefront)
        #   - batch b+1's qT/kT/v projection units (dense PE, independent)
        #   - batch b-1's P@V / normalize / proj (inputs all ready -> these
        #     fill every stall the exp wavefront would otherwise cause)
        # The projection of batch n-2 is held back as dependency-free PE
        # filler for the drain; ~60 junk weight-loads keep the PE clock
        # ramped through the initial DMA wait.
        qv_units, state = qkv_units(0, boost=True)
        for u in qv_units:
            u()
        prefetch(3)
        held_proj = None
        sc_last = None
        pTs_last = None
        tail = []          # av/norm/proj units of batch b-1
        for b in range(n_batches):
            attnT = [aT_p.tile([128, N], bf16, tag=f"aT{j}", name=f"aT{j}")
                     for j in range(6)]
            dens = rc_p.tile([128, N], f32, tag="dens", name="dens")
            dstage = rc_p.tile([128, 3 * N], f32, tag="dstage", name="dstage")
            denf = rc_p.tile([128, N], f32, tag="denf", name="denf")
            den_r = rc_p.tile([128, N], bf16, tag="den_r", name="den_r")
            pTs = [[None] * 3 for _ in range(H)]
            sc = score_units(b, state, pTs)
            if b + 1 < n_batches:
                qv, nstate = qkv_units(b + 1)
            else:
                qv, nstate = [], None
            ns, nq, nt = len(sc), len(qv), len(tail)
            for i in range(max(ns, nq, nt)):
                if i < nt:
                    tail[i]()
                if i < ns:
                    sc[i]()
                if i < nq:
                    qv[i]()
            if b == n_batches - 1 and n_batches > 1:
                tail = last_tail_units(b, state, pTs, attnT, dstage,
                                       held_proj)
            else:
                units = (av_units(b, state, pTs, attnT, dens, dstage)
                         + norm_proj_units(b, attnT, dens, denf, den_r))
                if b == n_batches - 2:
                    # hold this batch's projection back: it becomes the
                    # dependency-free PE filler for the drain
                    tail = units[:-6]
                    held_proj = _mk_held_factory(attnT, b)
                else:
                    tail = units
            state = nstate
            prefetch(b + 4)
        for u in tail:
            u()

    nc.compile()
    return nc


def prep_host(x, qkv_w, qkv_b, proj_w, proj_b, rpb_table, rel_index):
    """Host-side preprocessing: fold scale/gather/exp/transposes + fp8
    quantization of the q path."""
    scale = d ** -0.5
    qkv_w = np.asarray(qkv_w, np.float32)
    shared = {}
    if USE_FP8:
        wq = qkv_w[:, :DIM] * (scale * SWQ)
        np.clip(wq, -224.0, 224.0, out=wq)
        shared["wq8"] = np.ascontiguousarray(
            wq.astype(_E4M3).reshape(3, 2, 128, KQ).transpose(0, 2, 1, 3)
        ).reshape(384, 2, KQ)
        shared["w_k"] = np.ascontiguousarray(
            qkv_w[:, DIM:2 * DIM]).astype(_BF16)
    else:
        shared["w_k"] = np.ascontiguousarray(np.concatenate(
            [qkv_w[:, :DIM] * scale, qkv_w[:, DIM:2 * DIM]], axis=1)
        ).astype(_BF16)
    shared["w_v"] = np.ascontiguousarray(qkv_w[:, 2 * DIM:]).astype(_BF16)
    shared["proj_w"] = np.asarray(proj_w, np.float32).astype(_BF16)
    bias = np.asarray(rpb_table)[:, np.asarray(rel_index)]       # [H, nq, nk]
    expB = np.exp(bias.transpose(0, 2, 1)).astype(_BF16)          # [H, nk, nq]
    eBp = np.zeros((H, 384, N), dtype=_BF16)
    eBp[:, :N, :] = expB
    shared["expB"] = np.ascontiguousarray(
        eBp.reshape(H, 3, 128, N).transpose(0, 2, 1, 3))
    qkv_b = np.asarray(qkv_b, np.float32)
    qsc = scale * SX * SWQ if USE_FP8 else scale
    qkb = np.concatenate([qkv_b[:DIM] * qsc, qkv_b[DIM:2 * DIM]])
    vb = qkv_b[2 * DIM:]
    has_qkb = bool(np.any(qkb))
    has_vb = bool(np.any(vb))
    has_pb = bool(np.any(np.asarray(proj_b)))

    ind = np.zeros((H, DIM), dtype=_BF16)
    for h in range(H):
        ind[h, h * 64:(h + 1) * 64] = 1.0
    # ind4[r, par*128 + m] for the last batch's per-group broadcast:
    # pair j (parity par=j%2) takes den rows 2*par + (m>=64).
    ind4 = np.zeros((4, 256), dtype=_BF16)
    for par in range(2):
        ind4[2 * par, par * 128:par * 128 + 64] = 1.0
        ind4[2 * par + 1, par * 128 + 64:par * 128 + 128] = 1.0
    shared["ind"] = ind
    shared["ind4"] = ind4
    if has_qkb:
        shared["qkb"] = np.ascontiguousarray(qkb.reshape(12, 128)).astype(np.float32)
    if has_vb:
        shared["vb"] = vb.reshape(1, DIM).astype(_BF16)
    if has_pb:
        shared["pb"] = np.asarray(proj_b).reshape(1, DIM).astype(_BF16)

    in_maps = []
    for c in range(N_CORES):
        xs = np.asarray(x[c * B_LOC:(c + 1) * B_LOC], np.float32)
        xf = xs.reshape(B_LOC * N, DIM)
        xT = np.ascontiguousarray(xf.T).astype(_BF16)
        m = {"xT": xT}
        if USE_FP8:
            xq = xf.T * SX
            np.clip(xq, -224.0, 224.0, out=xq)
            m["x8"] = np.ascontiguousarray(
                xq.astype(_E4M3).reshape(3, 2, 128, B_LOC * N)
                .transpose(0, 2, 1, 3)).reshape(384, 2, B_LOC * N)
        m.update(shared)
        in_maps.append(m)
    return in_maps, has_qkb, has_vb, has_pb


_NC_CACHE = {}


def kernel(x, qkv_w, qkv_b, proj_w, proj_b, rpb_table, rel_index):
    from concourse.bass_utils import run_bass_kernel_spmd

    in_maps, has_qkb, has_vb, has_pb = prep_host(
        x, qkv_w, qkv_b, proj_w, proj_b, rpb_table, rel_index)
    key = (has_qkb, has_vb, has_pb, USE_FP8)
    if key not in _NC_CACHE:
        _NC_CACHE[key] = build_nc(B_LOC, has_qkb, has_vb, has_pb)
    nc = _NC_CACHE[key]
    res = run_bass_kernel_spmd(nc, in_maps, core_ids=list(range(N_CORES)))
    out = np.concatenate(
        [np.asarray(res.results[c]["out"], dtype=np.float32).T.reshape(
            B_LOC, N, DIM) for c in range(N_CORES)],
        axis=0)
    return out

